# revision 74
# baseline (speedup 1.0000x reference)
# Trainium2 Bass kernel for a causal multi-head attention block.
#
# Reference computation (fp32):
#   qkv = x @ w_attn + b_attn ; split into q,k,v heads (N=16, H=64)
#   scores = q @ k^T / sqrt(H), causal mask, softmax over keys
#   out = (weights @ v) reshaped, then out @ w_proj + b_proj
#
# Sharding: 8 cores = 2 batches x 4 head-groups (4 heads each).
#   - batch data-parallel, heads tensor-parallel (c_attn columns / c_proj rows)
#   - each core emits a partial [T, D] projection output; host sums the 4
#     head-group partials per batch and adds b_proj (the gather step).
#
# Key design points vs a naive port (exp on the ACT engine is the hard
# roofline at ~73us busy; everything else is scheduled around keeping its
# stream saturated):
#   - q/k are computed with fp8e4m3 DoubleRow matmuls (0.5 cyc/row) from a
#     host-pretransposed, D-chunk-pair-packed fp8 copy of x^T; scores also
#     run fp8 DoubleRow by producing q^T/k^T directly in the packed
#     [32 partitions, (head-dim parity, token)] layout via a host-side
#     wq/wk column permutation (the packing costs zero extra PE cycles).
#     V and the projection stay bf16 (1 cyc/row) for accuracy.
#   - scores are computed TRANSPOSED (S^T[s,t]) so exp(S^T) tiles feed
#     the weights@V matmul directly (contraction over s = partition dim);
#     row sums come free via a ones-column in V.
#   - emission is software-pipelined: each (ACT-bound) attention block
#     interleaves the next block's q/k/V matmul quanta and earlier blocks'
#     projection quanta between its s-tiles, scores run one s-tile ahead
#     of the AV consumers (carried across block boundaries), and the last
#     block's projection is folded behind its per-tile normalizes so the
#     post-exp tail is just one 128-row chain.
#   - psum drains avoid GPSIMD (no PSUM access); they split across DVE and
#     the ACT engine only where ACT is otherwise idle (startup, tail).
#   - y partials are stored as bf16 (halved DMA) and summed on the host.

import math

import numpy as np

B, T, D = 2, 2048, 1024
NHEAD, H = 16, 64
HPC = 4            # heads per core
CD = HPC * H       # 256 head-dim columns per core
N_CORES = 8
P = 128            # partitions
TT = T // P        # 16 t-tiles of 128
TB = T // 512      # 4 t-blocks of 512
KD = D // P        # 8 contraction tiles over D
G = H + 2          # v columns per head (64 v + ones + zero)

_CACHE = {}

# scheduling knobs (swept via sweep.py; keys mirror code sites)
TUNE = {
    "act_drain_blocks": (0,),   # blocks whose 2nd qk drains go to ACT
    "warmup": 0,                # PE warm-up matmuls (no measurable effect)
    "frontload": {0: 2, 1: 2, 2: 2, 3: 2},  # per-tb pacing bonus
    "ysb_bufs": 4,
    "et_bufs": 20,
    "q0_act_drain": (lambda ih, th: th == 1),
    "k1_act_drains": False,
}


def _build_module(mm_dt_name: str):
    import contextlib

    import concourse.bass as bass  # noqa: F401
    import concourse.mybir as mybir
    import concourse.tile as tile
    from concourse import bacc

    f32 = mybir.dt.float32
    bf16 = mybir.dt.bfloat16
    fp8 = mybir.dt.float8e4
    DR = mybir.MatmulPerfMode.DoubleRow

    nc = bacc.Bacc("TRN2", target_bir_lowering=False, debug=False)

    # x8: host-pretransposed x^T in fp8 DoubleRow pair layout
    # (p, j-block, kk-chunkpair, i-pair, t): val = x[512j+t, 256kk+128i+p]
    x8_d = nc.dram_tensor("x8", [P, TB * 4 * 2 * 512], fp8, kind="ExternalInput").ap()
    # xt16: host-pretransposed x^T in bf16, (p, j, k-chunk, t)
    xt16_d = nc.dram_tensor("xt16", [P, TB * KD * 512], bf16, kind="ExternalInput").ap()
    # wq8/wk8: fp8, (p, kk, i, col) with col order (parity-major, 32g+p')
    # so q^T/k^T psums come out in the scores' DoubleRow packed layout.
    wq8_d = nc.dram_tensor("wq8", [P, 4 * 2 * 2 * P], fp8, kind="ExternalInput").ap()
    wk8_d = nc.dram_tensor("wk8", [P, 4 * 2 * 2 * P], fp8, kind="ExternalInput").ap()
    wv_d = nc.dram_tensor("wv", [P, KD * CD], bf16, kind="ExternalInput").ap()
    wp_d = nc.dram_tensor("wp", [P, 2 * D], bf16, kind="ExternalInput").ap()
    bqk_d = nc.dram_tensor("bqk", [P, 4], f32, kind="ExternalInput").ap()
    bv_d = nc.dram_tensor("bv", [P, CD], bf16, kind="ExternalInput").ap()
    mask_d = nc.dram_tensor("mask", [P, P], bf16, kind="ExternalInput").ap()
    onescol_d = nc.dram_tensor("onescol", [P, 2 * HPC], bf16, kind="ExternalInput").ap()
    ident_d = nc.dram_tensor("ident", [P, P], bf16, kind="ExternalInput").ap()
    y_d = nc.dram_tensor("y", [T, D], bf16, kind="ExternalOutput").ap()

    with tile.TileContext(nc) as tc, contextlib.ExitStack() as ctx:
        const_p = ctx.enter_context(tc.tile_pool(name="const", bufs=1))
        w_p = ctx.enter_context(tc.tile_pool(name="weights", bufs=1))
        xt_p = ctx.enter_context(tc.tile_pool(name="xt", bufs=1))
        qkt_p = ctx.enter_context(tc.tile_pool(name="qkt", bufs=1))
        v_p = ctx.enter_context(tc.tile_pool(name="vbuf", bufs=1))
        e_p = ctx.enter_context(tc.tile_pool(name="epool", bufs=TUNE["et_bufs"]))
        attn_p = ctx.enter_context(tc.tile_pool(name="attn", bufs=1))
        at_p = ctx.enter_context(tc.tile_pool(name="atp", bufs=1))
        small_p = ctx.enter_context(tc.tile_pool(name="small", bufs=8))
        # PSUM, 8 banks:
        #   wps [128,512]f32 x2   (qk psums, v psums)             2 banks
        #   sp  [128,1024]f32 x2  (scores; proj y accum)          4 banks
        #   accp0/1 [128,264]f32  (AV accumulators)              ~2 banks
        psp = ctx.enter_context(tc.tile_pool(name="psp", bufs=2, space="PSUM"))

        # ---- tiles for constants / weights (DMAs emitted in the schedule
        # below, ordered by first use, interleaved with the block-0 x^T
        # transposes: the DMA queue is the startup critical path) ----
        wq8_sb = w_p.tile([P, 4 * 2 * 2 * P], fp8, name="wq8_sb")
        wk8_sb = w_p.tile([P, 4 * 2 * 2 * P], fp8, name="wk8_sb")
        bqk = const_p.tile([P, 4], f32, name="bqk_sb")
        wv_sb = w_p.tile([P, KD * CD], bf16, name="wv_sb")
        bv = const_p.tile([P, CD], bf16, name="bv_sb")
        onescol = const_p.tile([P, 2 * HPC], bf16, name="onescol_sb")
        mask = const_p.tile([P, P], bf16, name="mask_sb")
        wp_sb = w_p.tile([P, 2 * D], bf16, name="wp_sb")
        ident = const_p.tile([P, P], bf16, name="ident_sb")

        # persistent activation buffers
        qt_sb = [
            qkt_p.tile([P, 1024], fp8, name=f"qt{j}", tag=f"qt{j}")
            for j in range(TB)
        ]
        kt_sb = [
            qkt_p.tile([P, 1024], fp8, name=f"kt{j}", tag=f"kt{j}")
            for j in range(TB)
        ]
        v_sb = [
            v_p.tile([P, HPC * G], bf16, name=f"v{i}", tag=f"v{i}")
            for i in range(TT)
        ]
        # per-(head-pair, t-tile) attn tiles: the projection's transposes
        # unlock per 128-column slice as soon as its normalize lands
        # (dependency tracking is tile-granular)
        attn_t = {
            (tb, c, dj): attn_p.tile([P, P], bf16, name=f"attn{tb}_{c}_{dj}",
                                     tag=f"attn{tb}_{c}_{dj}")
            for tb in range(TB)
            for c in range(2)
            for dj in range(4)
        }

        # per-block x tiles (loaded with one plain DMA each from the
        # host-pretransposed layouts)
        x8_sb = [
            xt_p.tile([P, 4096], fp8, name=f"x8_{j}", tag=f"x8_{j}")
            for j in range(TB)
        ]
        xt16_sb = [
            xt_p.tile([P, 4096], bf16, name=f"xt16_{j}", tag=f"xt16_{j}")
            for j in range(TB)
        ]

        def load_x8(j, eng):
            eng.dma_start(x8_sb[j], x8_d[:, 4096 * j : 4096 * (j + 1)])

        def load_xt16(j, eng):
            eng.dma_start(xt16_sb[j], xt16_d[:, 4096 * j : 4096 * (j + 1)])

        def qk_quanta(j):
            """q^T/k^T for block j: fp8 DoubleRow over D-chunk pairs, psum
            partition = 32g+p', free half ih holds features h=2p'+ih of
            head g.  Yields 4 quanta of 4 matmuls (~430ns PE each); the two
            parity groups of q (then k) are emitted back-to-back with both
            drains after, so the PE doesn't sit behind a drain between
            parity groups.

            Block 0 (startup critical path): groups split into 256-wide
            halves spread over four independent psum slots (wps x2 plus the
            still-idle AV accumulator banks) with drains alternating
            DVE/ACT, removing the two-buffer drain-chain serialization."""
            w4s = [wq8_sb.rearrange("p (kk i c) -> p kk i c", kk=4, i=2),
                   wk8_sb.rearrange("p (kk i c) -> p kk i c", kk=4, i=2)]
            xr = x8_sb[j].rearrange("p (kk i t) -> p kk i t", kk=4, i=2)
            if j == 0:
                for qk in range(2):
                    w4 = w4s[qk]
                    dst = (qt_sb if qk == 0 else kt_sb)[j]
                    pss = []
                    for ih in range(2):
                        for th in range(2):
                            tags = ["wps", "wps", "accp0", "accp1"]
                            ps = psp.tile([P, 256], f32, name="qkp0",
                                          tag=tags[2 * ih + th],
                                          bufs=2 if ih == 0 else 1)
                            for kk in range(4):
                                nc.tensor.matmul(
                                    ps,
                                    w4[:, kk, :, P * ih : P * (ih + 1)],
                                    xr[:, kk, :, 256 * th : 256 * (th + 1)],
                                    start=(kk == 0),
                                    stop=(kk == 3),
                                    perf_mode=DR,
                                )
                            pss.append((ih, th, ps))
                        yield
                    for ih, th, ps in pss:
                        idx = 2 * qk + ih
                        dslc = dst[:, 512 * ih + 256 * th :
                                   512 * ih + 256 * (th + 1)]
                        if TUNE["q0_act_drain"](ih, th):
                            nc.scalar.activation(
                                dslc, ps,
                                mybir.ActivationFunctionType.Identity,
                                bias=bqk[:, idx : idx + 1],
                            )
                        else:
                            nc.vector.tensor_scalar_add(
                                dslc, ps, bqk[:, idx : idx + 1],
                            )
                return
            for qk in range(2):
                w4 = w4s[qk]
                dst = (qt_sb if qk == 0 else kt_sb)[j]
                pss = []
                for ih in range(2):
                    ps = psp.tile([P, 512], f32, name="qkp", tag="wps")
                    for kk in range(4):
                        nc.tensor.matmul(
                            ps,
                            w4[:, kk, :, P * ih : P * (ih + 1)],
                            xr[:, kk],
                            start=(kk == 0),
                            stop=(kk == 3),
                            perf_mode=DR,
                        )
                    pss.append(ps)
                    yield
                # psum->sbuf drains to fp8, per-partition bias folded.
                for ih in range(2):
                    idx = 2 * qk + ih
                    nc.vector.tensor_scalar_add(
                        dst[:, 512 * ih : 512 * (ih + 1)], pss[ih],
                        bqk[:, idx : idx + 1],
                    )

        def v_quanta(j, tis=(0, 1, 2, 3)):
            """V for block j; quanta of 8 narrow matmuls (~850ns PE)."""
            xr = xt16_sb[j].rearrange("p (k t) -> p k t", k=KD)
            for ti in tis:
                g = 4 * j + ti
                ps = psp.tile([P, CD], f32, name="vp", tag="wps")
                for k in range(KD):
                    nc.tensor.matmul(
                        ps,
                        xr[:, k, P * ti : P * (ti + 1)],
                        wv_sb[:, CD * k : CD * (k + 1)],
                        start=(k == 0),
                        stop=(k == KD - 1),
                    )
                # psum->sbuf drain with the (partition-broadcast) V bias
                vg = v_sb[g].rearrange("p (g c) -> p g c", g=HPC)
                nc.vector.tensor_add(
                    vg[:, :, 0:H],
                    ps.rearrange("p (g c) -> p g c", g=HPC),
                    bv.rearrange("p (g c) -> p g c", g=HPC),
                )
                nc.gpsimd.tensor_copy(
                    vg[:, :, H : H + 2],
                    onescol.rearrange("p (g c) -> p g c", c=2),
                )
                yield

        attnT_store = {}

        def proj_T(jb, c, ti):
            """attn^T PE transpose + drain for head-pair c, t-tile ti."""
            pt = psp.tile([P, P], bf16, name="atp", tag="wps")
            nc.tensor.transpose(pt, attn_t[(jb, c, ti)], ident)
            tag = f"at3_{c}_{ti}" if jb == 3 else f"at{c}_{ti}"
            at = at_p.tile([P, P], bf16, name=tag, tag=tag,
                           bufs=1 if jb == 3 else 2)
            nc.vector.tensor_copy(at, pt)
            attnT_store[(jb, c, ti)] = at

        def proj_jl(jb, jl):
            """y rows [128*(4jb+jl)] = attn^T @ wp, drained to sbuf + DMA."""
            jt = 4 * jb + jl
            ysb = small_p.tile([P, 1024], bf16, name="ysb", tag="ysb",
                               bufs=TUNE["ysb_bufs"])
            for n in range(2):
                ps = psp.tile([P, 512], f32, name="yp", tag="wps")
                for c in range(2):
                    nc.tensor.matmul(
                        ps,
                        attnT_store[(jb, c, jl)],
                        wp_sb[:, D * c + 512 * n : D * c + 512 * (n + 1)],
                        start=(c == 0),
                        stop=(c == 1),
                    )
                dst = ysb[:, 512 * n : 512 * (n + 1)]
                if jb == 3 and n == 1:
                    nc.scalar.copy(dst, ps)
                else:
                    nc.vector.tensor_copy(dst, ps)
            (nc.sync if jl % 2 == 0 else nc.scalar).dma_start(
                y_d[P * jt : P * (jt + 1), :], ysb
            )

        def proj_quanta(jb):
            """attn^T transposes + y = attn @ wp for t-block jb; 8 quanta."""
            for c in range(2):
                if (jb, c, 0) not in attnT_store:
                    for half in range(2):
                        proj_T(jb, c, 2 * half)
                        proj_T(jb, c, 2 * half + 1)
                        yield
            for jl in range(4):
                proj_jl(jb, jl)
                yield

        def attention(tb, fillers, prev_pend=None, mid_hook=None,
                      post_nrm=None, v_force=(1, 2, 3, 4)):
            """S^T -> exp -> AV for 512-wide t-block tb, heads processed in
            pairs; one psum tile holds both heads' scores so a single exp
            covers both.  `fillers` is a list of generators whose quanta
            (independent PE work) are interleaved between s-tiles to keep
            the PE busy while ACT streams the exps."""
            import itertools

            fill_iter = itertools.chain(*fillers)
            n_quanta = 0
            total_st = 2 * (4 * tb + 4)
            total_fill = {0: 8, 1: 8, 2: 16, 3: 20}[tb]
            st_idx = 0

            _done = object()

            def pull_fillers(force=None):
                nonlocal n_quanta
                want = (st_idx + 1) * (total_fill + TUNE["frontload"][tb]) // total_st
                if force is not None:
                    want = max(want, force)
                want = min(total_fill, want)
                while n_quanta < want:
                    if next(fill_iter, _done) is _done:
                        n_quanta = total_fill
                        break
                    n_quanta += 1

            def emit_scores(hp, i):
                """scores + exp for s-tile i of head-pair hp; returns the
                exp tile (both heads side by side)."""
                first = max(0, i - 4 * tb)
                c0 = P * first
                sps = psp.tile([P, 1024], f32, name="sp", tag="sp", bufs=2)
                for hh in range(2):
                    g = 2 * hp + hh
                    lhsT = kt_sb[i // 4][32 * g : 32 * (g + 1), :].rearrange(
                        "p (i t) -> p i t", i=2
                    )[:, :, P * (i % 4) : P * (i % 4 + 1)]
                    rhs = qt_sb[tb][32 * g : 32 * (g + 1), :].rearrange(
                        "p (i t) -> p i t", i=2
                    )[:, :, c0:512]
                    nc.tensor.matmul(
                        sps[:, 512 * hh + c0 : 512 * hh + 512],
                        lhsT,
                        rhs,
                        start=True,
                        stop=True,
                        perf_mode=DR,
                        tile_position=(32 * g, 0),
                    )
                et = e_p.tile([P, 1024], bf16, name="et", tag="et")
                if first:
                    nc.scalar.activation(
                        et.rearrange("p (g c) -> p g c", g=2)[:, :, c0:512],
                        sps.rearrange("p (g c) -> p g c", g=2)[:, :, c0:512],
                        mybir.ActivationFunctionType.Exp,
                        scale=1.0 / math.sqrt(H),
                    )
                else:
                    nc.scalar.activation(
                        et, sps,
                        mybir.ActivationFunctionType.Exp,
                        scale=1.0 / math.sqrt(H),
                    )
                return et

            def emit_av(hp, i, et, acc_t, post_nrm=None):
                first = max(0, i - 4 * tb)
                dj = i - 4 * tb  # diagonal jj of this s-tile, if any
                etd = None
                if 0 <= dj <= 3:
                    # masked diagonal sub-tiles go to a separate tile so
                    # the non-diagonal AV matmuls don't serialize behind
                    # the mask write (tile-granular dependency tracking)
                    etd = e_p.tile([P, 2 * P], bf16, name="etd", tag="etd",
                                   bufs=2)
                    for hh in range(2):
                        nc.gpsimd.tensor_mul(
                            etd[:, P * hh : P * (hh + 1)],
                            et[:, 512 * hh + P * dj : 512 * hh + P * (dj + 1)],
                            mask,
                        )
                for jj in range(first, 4):
                    jglob = 4 * tb + jj
                    for hh in range(2):
                        if jj == dj:
                            lhs_e = etd[:, P * hh : P * (hh + 1)]
                        else:
                            lhs_e = et[
                                :, 512 * hh + P * jj : 512 * hh + P * (jj + 1)
                            ]
                        # start=True clears has_written for the WHOLE
                        # psum bank: only the first group per bank
                        # issues it.
                        nc.tensor.matmul(
                            acc_t[hh][:, 66 * jj : 66 * jj + 66],
                            lhs_e,
                            v_sb[i][:, G * (2 * hp + hh) : G * (2 * hp + hh) + 66],
                            start=(i == 0 and jj == 0),
                            stop=(i == jglob),
                            skip_group_check=True,
                        )
                if 0 <= dj <= 3:
                    # acc group dj just received its last (diagonal)
                    # contribution: normalize it now so the psum bank
                    # region drains while later s-tiles still accumulate
                    for hh in range(2):
                        s0 = 66 * dj
                        rec = small_p.tile([P, 1], f32, name="rec",
                                           tag="rec")
                        nc.vector.reciprocal(
                            rec, acc_t[hh][:, s0 + H : s0 + H + 1]
                        )
                        if tb == 3 and hp == 1:
                            # tail: ACT is idle once the last exps are out
                            nc.scalar.activation(
                                attn_t[(tb, hp, dj)][:, H * hh : H * (hh + 1)],
                                acc_t[hh][:, s0 : s0 + H],
                                mybir.ActivationFunctionType.Copy,
                                scale=rec,
                            )
                        else:
                            nc.vector.tensor_scalar_mul(
                                attn_t[(tb, hp, dj)][:, H * hh : H * (hh + 1)],
                                acc_t[hh][:, s0 : s0 + H],
                                rec,
                            )
                    if post_nrm is not None:
                        post_nrm(hp, dj)

            # 1-deep software pipeline (carried across attention calls):
            # scores(i+1)/exp(i+1) are emitted before AV(i), so the PE keeps
            # the ACT exp stream fed one tile ahead instead of blocking on
            # exp(i) for AV(i).
            n_s = 4 * tb + 4  # s-tiles 0 .. 4*tb+3
            pend = prev_pend  # (emit_fn, is_last_of_hp0, v_deadline)
            for hp in range(2):
                acc_t = [
                    psp.tile([P, 4 * 66], f32, name="accp", tag=f"accp{a}",
                             bufs=1)
                    for a in range(2)
                ]
                for i in range(n_s):
                    et = emit_scores(hp, i)
                    if pend is not None:
                        fn, was_last_of_hp0, vdl = pend
                        if vdl is not None:
                            pull_fillers(force=vdl)
                        fn()
                        if was_last_of_hp0 and mid_hook is not None:
                            mid_hook()
                            mid_hook = None
                    pend = (
                        (lambda hp=hp, i=i, et=et, acc_t=acc_t:
                         emit_av(hp, i, et, acc_t, post_nrm=post_nrm)),
                        hp == 0 and i == n_s - 1,
                        v_force[i - 4 * tb] if i >= 4 * tb else None,
                    )
                    st_idx += 1
                    pull_fillers()
            # flush any remaining filler quanta
            for _ in fill_iter:
                pass
            return pend

        # ---- startup DMA schedule, ordered by first use (the single DMA
        # pipe + ~630ns/dispatch HWDGE are the startup critical path).
        # Block-0 loads are split so the first consumers unblock early. ----
        nc.sync.dma_start(wq8_sb, wq8_d)
        load_x8(0, nc.scalar)
        nc.sync.dma_start(wk8_sb, wk8_d)
        nc.scalar.dma_start(bqk, bqk_d)
        nc.sync.dma_start(wv_sb, wv_d)
        load_xt16(0, nc.scalar)
        nc.sync.dma_start(mask, mask_d)
        nc.sync.dma_start(onescol, onescol_d)
        nc.sync.dma_start(bv, bv_d)
        load_x8(1, nc.sync)
        load_xt16(1, nc.scalar)
        nc.sync.dma_start(ident, ident_d)
        nc.scalar.dma_start(wp_sb, wp_d)

        # warm the ACT exp table + PE clock ramp while the DMAs stream in
        warm = small_p.tile([P, 1], f32, name="warm", tag="warm")
        nc.scalar.activation(warm, bqk[:, 0:1],
                             mybir.ActivationFunctionType.Exp)
        wsrc = small_p.tile([P, 256], bf16, name="wsrc", tag="wsrc")
        nc.vector.memset(wsrc, 0.0)
        for r in range(TUNE["warmup"]):
            wps = psp.tile([P, 256], f32, name="wmm", tag="accp0", bufs=1)
            nc.tensor.matmul(wps, wsrc[:, 0:P], wsrc, start=True, stop=True)

        # software-pipelined emission: attention(tb) (ACT-bound) interleaves
        # this block's V and the next block's q/k quanta (PE-bound) between
        # its s-tiles; later attentions interleave earlier projection
        # blocks; the only non-overlapped tail is projection(3).
        for _ in qk_quanta(0):
            pass
        pend = attention(0, [v_quanta(0), qk_quanta(1)])
        load_x8(2, nc.sync)
        load_xt16(2, nc.scalar)
        pend = attention(1, [v_quanta(1), qk_quanta(2)], prev_pend=pend)
        load_x8(3, nc.sync)
        load_xt16(3, nc.scalar)
        pend = attention(2, [v_quanta(2), qk_quanta(3), proj_quanta(0)],
                         prev_pend=pend)
        def proj3_hook(hp, dj):
            if hp == 1:
                proj_T(3, 1, dj)
                proj_jl(3, dj)

        pend = attention(3, [v_quanta(3), proj_quanta(1), proj_quanta(2)],
                         prev_pend=pend,
                         mid_hook=lambda: [proj_T(3, 0, ti) for ti in range(4)],
                         post_nrm=proj3_hook)
        pend[0]()

    nc.compile()
    return nc


def _get_module(mm_dt_name: str):
    if mm_dt_name not in _CACHE:
        _CACHE[mm_dt_name] = _build_module(mm_dt_name)
    return _CACHE[mm_dt_name]


def kernel(x, w_attn, b_attn, w_proj, b_proj, mm_dt_name: str = "float32r",
           trace: bool = False):
    from concourse.bass_utils import run_bass_kernel_spmd

    x = np.asarray(x, dtype=np.float32)
    w_attn = np.asarray(w_attn, dtype=np.float32)
    b_attn = np.asarray(b_attn, dtype=np.float32)
    w_proj = np.asarray(w_proj, dtype=np.float32)
    b_proj = np.asarray(b_proj, dtype=np.float32)

    nc = _get_module(mm_dt_name)

    import ml_dtypes

    bf = np.dtype(ml_dtypes.bfloat16)
    f8 = np.dtype(ml_dtypes.float8_e4m3)
    mask = np.triu(np.ones((P, P), dtype=bf))

    # packed column order for wq/wk: column (ih*128 + 32g + p') holds
    # feature 64g + 2p' + ih of the head group.
    perm = np.zeros(2 * P, dtype=np.int64)
    for ih in range(2):
        for g in range(HPC):
            for pp in range(32):
                perm[ih * P + 32 * g + pp] = 64 * g + 2 * pp + ih

    def chunked(w):  # [D, C] -> [128, KD*C] with chunk k at cols [k*C,(k+1)*C)
        Dm, C = w.shape
        return np.ascontiguousarray(
            w.reshape(KD, P, C).transpose(1, 0, 2).reshape(P, KD * C)
        )

    def pack_w8(w):  # [D, 256] -> [128, (kk, i, c)] fp8
        return np.ascontiguousarray(
            w.reshape(4, 2, P, 2 * P).transpose(2, 0, 1, 3).reshape(P, -1)
        ).astype(f8)

    in_maps = []
    for core in range(N_CORES):
        b = core // 4
        gcore = core % 4
        c0 = CD * gcore
        wq = w_attn[:, c0 : c0 + CD][:, perm]
        wk = w_attn[:, D + c0 : D + c0 + CD][:, perm]
        wv = w_attn[:, 2 * D + c0 : 2 * D + c0 + CD]
        bq = b_attn[c0 : c0 + CD][perm]
        bk = b_attn[D + c0 : D + c0 + CD][perm]
        bvv = b_attn[2 * D + c0 : 2 * D + c0 + CD]
        # bqk columns: (q i=0, q i=1, k i=0, k i=1), partition = 32g+p'
        bqk = np.stack(
            [bq[0:P], bq[P : 2 * P], bk[0:P], bk[P : 2 * P]], axis=1
        ).astype(np.float32)
        wp = w_proj[c0 : c0 + CD, :]  # [256, 1024]
        xt = np.ascontiguousarray(x[b].T)  # [D, T]
        # x8: (p, j, kk, i, t) = xt[256kk+128i+p, 512j+t], fp8
        x8 = (
            xt.reshape(4, 2, P, TB, 512)
            .transpose(2, 3, 0, 1, 4)
            .reshape(P, -1)
            .astype(f8)
        )
        # xt16: (p, j, k, t) = xt[128k+p, 512j+t], bf16
        xt16 = (
            xt.reshape(KD, P, TB, 512)
            .transpose(1, 2, 0, 3)
            .reshape(P, -1)
            .astype(bf)
        )
        in_maps.append(
            {
                "x8": np.ascontiguousarray(x8),
                "xt16": np.ascontiguousarray(xt16),
                "wq8": pack_w8(wq),
                "wk8": pack_w8(wk),
                "wv": chunked(wv).astype(bf),
                "wp": np.ascontiguousarray(
                    wp.reshape(2, P, D).transpose(1, 0, 2).reshape(P, 2 * D)
                ).astype(bf),
                "bqk": bqk,
                "bv": np.broadcast_to(bvv[None, :], (P, CD)).astype(bf).copy(),
                "mask": mask,
                "onescol": np.tile(np.array([1.0, 0.0], bf), (P, HPC)),
                "ident": np.eye(P, dtype=bf),
            }
        )

    res = run_bass_kernel_spmd(
        nc, in_maps, core_ids=list(range(N_CORES)), trace=trace
    )

    out = np.zeros((B, T, D), dtype=np.float32)
    for core in range(N_CORES):
        out[core // 4] += res.results[core]["y"].astype(np.float32)
    out += b_proj[None, None, :]
    if trace:
        kernel.last_result = res
    return out
